# revision 7
# baseline (speedup 1.0000x reference)
"""Trainium2 Bass kernel for DecoderAttention (b=2, n=2048, m=1024, d=1024, h=16).

Sharding: 8 cores = 2 (batch) x 4 (head groups of 4 heads).  Each core:
  - projects q/k/v for its 4 heads from x|context (pre-transposed on host),
  - runs causal flash attention in scores-transposed layout [kj, qi]
    (softmax without max subtraction -- scores are bounded; causally masked
    entries multiply to exactly 0 after exp, matching exp(-50000)),
  - computes its partial out-projection  attn_out_g @ Wo[rows_g]  [2048, 1024].
Host sums the 4 head-group partials per batch (the "all-reduce") and adds bo.

All matmuls run in bf16 with f32 PSUM accumulation (validated ~0.4% rel err).
"""

import os

# The neuron/axon jax backend must be discoverable for the PJRT execution
# path; a JAX_PLATFORMS=cpu pin (used when running the jax reference) would
# hide the trn2 devices from this process.
if os.environ.get("JAX_PLATFORMS", "").strip().lower() == "cpu":
    del os.environ["JAX_PLATFORMS"]

from contextlib import ExitStack

import ml_dtypes
import numpy as np

import concourse.bass as bass
import concourse.tile as tile
from concourse import bacc, mybir
from concourse.bass_utils import run_bass_kernel_spmd

B, N, M, D = 2, 2048, 1024, 1024
H, DH = 16, 64
NM = N + M          # 3072 keys (self + context)
GROUPS = 4          # head groups; 4 heads = 256 cols per group
GC = 256            # columns per head group
NCORES = 8
SCALE = DH ** -0.5
P = 128
KT = D // P         # 8 contraction tiles over d
QCH = 512           # query-chunk width
NQC = N // QCH      # 4 query chunks
NKJ = NM // P       # 24 key tiles
NSELF = N // P      # 16 self key tiles
FP32 = mybir.dt.float32
F32R = mybir.dt.float32r
BF16 = mybir.dt.bfloat16
BF16NP = ml_dtypes.bfloat16


def _active_kj(c):
    """Key tiles with any unmasked entry for query chunk c (512 queries)."""
    return list(range(0, 4 * c + 4)) + list(range(NSELF, NKJ))


def _build_module(biased: bool):
    nc = bacc.Bacc(
        "TRN2",
        target_bir_lowering=False,
        debug=False,
        enable_asserts=False,
        num_devices=NCORES,
    )
    xkvT_d = nc.dram_tensor("xkvT", [D, NM], BF16, kind="ExternalInput").ap()
    wq_d = nc.dram_tensor("wq", [D, GC], BF16, kind="ExternalInput").ap()
    wk_d = nc.dram_tensor("wk", [D, GC], BF16, kind="ExternalInput").ap()
    wv_d = nc.dram_tensor("wv", [D, GC], BF16, kind="ExternalInput").ap()
    wo_d = nc.dram_tensor("wo", [GC, D], BF16, kind="ExternalInput").ap()
    msk_d = nc.dram_tensor("msk", [4 * P, QCH], BF16, kind="ExternalInput").ap()
    if biased:
        bq_d = nc.dram_tensor("bq", [1, GC], BF16, kind="ExternalInput").ap()
        bk_d = nc.dram_tensor("bk", [1, GC], BF16, kind="ExternalInput").ap()
        bv_d = nc.dram_tensor("bv", [1, GC], BF16, kind="ExternalInput").ap()
    out_d = nc.dram_tensor("out", [N, D], FP32, kind="ExternalOutput").ap()

    with tile.TileContext(nc) as tc, ExitStack() as ctx:
        const = ctx.enter_context(tc.tile_pool(name="const", bufs=1))
        pexp = ctx.enter_context(tc.tile_pool(name="pexp", bufs=6))
        bcp = ctx.enter_context(tc.tile_pool(name="bcp", bufs=3))
        # PSUM budget: 8 banks = proj(2) + scores/bc(4, shared tag) + av(2)
        ps_main = ctx.enter_context(tc.tile_pool(name="ps_main", bufs=2, space="PSUM"))
        ps_s = ctx.enter_context(tc.tile_pool(name="ps_s", bufs=4, space="PSUM"))
        ps_av = ctx.enter_context(tc.tile_pool(name="ps_av", bufs=2, space="PSUM"))

        # ---- persistent SBUF tensors (column-concatenated k-tiles) ----
        xk = const.tile([P, KT * NM], BF16)          # xkvT: 8 tiles of [128, 3072]
        wqs = const.tile([P, KT * GC], BF16)
        wks = const.tile([P, KT * GC], BF16)
        wvs = const.tile([P, KT * GC], BF16)
        wos = const.tile([P, 2 * D], BF16)           # Wo rows: 2 tiles of [128, 1024]
        mks = const.tile([P, 4 * QCH], BF16)         # 4 diagonal mask tiles
        qT = const.tile([P, 2 * N], BF16)            # [head-pair cols, qi]
        kT = const.tile([P, 2 * NM], BF16)           # [head-pair cols, kj]
        vv = const.tile([P, NKJ * 4 * 65], BF16)     # per kj tile: 4x [v(64)|1]
        aT = const.tile([P, 2 * N], BF16)            # attn_out^T, 2 k-tiles
        ones_l = const.tile([1, 64], FP32)
        if biased:
            bq_s = const.tile([1, GC], BF16)
            bk_s = const.tile([1, GC], BF16)
            bv_s = const.tile([1, GC], BF16)
            ones_row = const.tile([1, QCH], BF16)
            ones_col = const.tile([1, P], BF16)

        # ---- input DMAs ----
        for kt in range(KT):
            nc.sync.dma_start(
                xk[:, kt * NM:(kt + 1) * NM], xkvT_d[kt * P:(kt + 1) * P, :]
            )
            nc.sync.dma_start(
                wqs[:, kt * GC:(kt + 1) * GC], wq_d[kt * P:(kt + 1) * P, :]
            )
            nc.sync.dma_start(
                wks[:, kt * GC:(kt + 1) * GC], wk_d[kt * P:(kt + 1) * P, :]
            )
            nc.sync.dma_start(
                wvs[:, kt * GC:(kt + 1) * GC], wv_d[kt * P:(kt + 1) * P, :]
            )
        for kt in range(2):
            nc.sync.dma_start(wos[:, kt * D:(kt + 1) * D], wo_d[kt * P:(kt + 1) * P, :])
        nc.sync.dma_start(
            mks.rearrange("p (t q) -> p t q", t=4),
            msk_d.rearrange("(t p) q -> p t q", p=P),
        )
        nc.vector.memset(ones_l[:], 1.0)
        if biased:
            nc.sync.dma_start(bq_s[:], bq_d[:])
            nc.sync.dma_start(bk_s[:], bk_d[:])
            nc.sync.dma_start(bv_s[:], bv_d[:])
            nc.vector.memset(ones_row[:], 1.0)
            nc.vector.memset(ones_col[:], 1.0)
        # ones columns interleaved into vv: col (t*260 + h*65 + 64)
        nc.gpsimd.memset(
            vv.rearrange("p (t h x) -> p t h x", t=NKJ, h=4)[:, :, :, 64:65], 1.0
        )

        # ---- projections ----
        # qT[mt]: [128, 2048] = (Wq cols mt).T @ xkvT[:, :N]  (pre-scaled on host)
        for mt in range(2):
            for c in range(NQC):
                psq = ps_main.tile([P, QCH], FP32, tag="proj")
                for kt in range(KT):
                    nc.tensor.matmul(
                        psq[:],
                        lhsT=wqs[:, kt * GC + mt * P: kt * GC + (mt + 1) * P],
                        rhs=xk[:, kt * NM + c * QCH: kt * NM + (c + 1) * QCH],
                        start=(kt == 0),
                        stop=(kt == KT - 1) and not biased,
                    )
                if biased:
                    nc.tensor.matmul(
                        psq[:],
                        lhsT=bq_s[:, mt * P:(mt + 1) * P],
                        rhs=ones_row[:],
                        start=False,
                        stop=True,
                    )
                nc.vector.tensor_copy(
                    qT[:, mt * N + c * QCH: mt * N + (c + 1) * QCH], psq[:]
                )
        # kT[mt]: [128, 3072]
        for mt in range(2):
            for c in range(NM // QCH):
                psk = ps_main.tile([P, QCH], FP32, tag="proj")
                for kt in range(KT):
                    nc.tensor.matmul(
                        psk[:],
                        lhsT=wks[:, kt * GC + mt * P: kt * GC + (mt + 1) * P],
                        rhs=xk[:, kt * NM + c * QCH: kt * NM + (c + 1) * QCH],
                        start=(kt == 0),
                        stop=(kt == KT - 1) and not biased,
                    )
                if biased:
                    nc.tensor.matmul(
                        psk[:],
                        lhsT=bk_s[:, mt * P:(mt + 1) * P],
                        rhs=ones_row[:],
                        start=False,
                        stop=True,
                    )
                nc.vector.tensor_copy(
                    kT[:, mt * NM + c * QCH: mt * NM + (c + 1) * QCH], psk[:]
                )
        # v natural layout per kj tile: [128 rows, 256 cols] -> interleaved [v|1]
        for t in range(NKJ):
            psv = ps_main.tile([P, GC], FP32, tag="proj")
            for kt in range(KT):
                nc.tensor.matmul(
                    psv[:],
                    lhsT=xk[:, kt * NM + t * P: kt * NM + (t + 1) * P],
                    rhs=wvs[:, kt * GC:(kt + 1) * GC],
                    start=(kt == 0),
                    stop=(kt == KT - 1) and not biased,
                )
            if biased:
                nc.tensor.matmul(
                    psv[:],
                    lhsT=ones_col[:],
                    rhs=bv_s[:],
                    start=False,
                    stop=True,
                )
            nc.vector.tensor_copy(
                vv[:, t * 260:(t + 1) * 260].rearrange("p (h x) -> p h x", h=4)[
                    :, :, 0:64
                ],
                psv.rearrange("p (h x) -> p h x", h=4),
            )

        # ---- attention (scores/exp one key-tile ahead of the AV matmul) ----
        for c in range(NQC):
            kjs = _active_kj(c)
            last = len(kjs) - 1
            for pair in range(2):
                ps_acc = [None, None]
                pending = None  # (p_tiles, i) exp'd tiles not yet fed to AV

                def do_av(p_tiles, i):
                    for hh in range(2):
                        h = pair * 2 + hh
                        t = kjs[i]
                        nc.tensor.matmul(
                            ps_acc[hh][:],
                            lhsT=vv[:, t * 260 + h * 65: t * 260 + (h + 1) * 65],
                            rhs=p_tiles[hh][:],
                            start=(i == 0),
                            stop=(i == last),
                        )

                for i, t in enumerate(kjs):
                    p_tiles = [None, None]
                    for hh in range(2):
                        lo, hi = hh * 64, hh * 64 + 64
                        pss = ps_s.tile([P, QCH], FP32, tag="s")
                        nc.tensor.matmul(
                            pss[:],
                            lhsT=kT[lo:hi, pair * NM + t * P: pair * NM + (t + 1) * P],
                            rhs=qT[lo:hi, pair * N + c * QCH: pair * N + (c + 1) * QCH],
                            start=True,
                            stop=True,
                        )
                        pt = pexp.tile([P, QCH], BF16, tag="p")
                        nc.scalar.activation(
                            pt[:], pss[:], mybir.ActivationFunctionType.Exp
                        )
                        if 4 * c <= t < 4 * c + 4:  # diagonal tile: causal mask
                            dt = t - 4 * c
                            nc.vector.tensor_mul(
                                pt[:], pt[:], mks[:, dt * QCH:(dt + 1) * QCH]
                            )
                        p_tiles[hh] = pt
                    if i == 0:
                        ps_acc[0] = ps_av.tile([65, QCH], FP32, tag="av", name="av0")
                        ps_acc[1] = ps_av.tile([65, QCH], FP32, tag="av", name="av1")
                    if pending is not None:
                        do_av(*pending)
                    pending = (p_tiles, i)
                do_av(*pending)

                # normalize: rows 0..63 / row 64, write into aT
                for hh in range(2):
                    h = pair * 2 + hh
                    acc = ps_acc[hh]
                    rec = bcp.tile([1, QCH], FP32, tag="rec")
                    nc.vector.reciprocal(rec[:], acc[64:65, :])
                    psb = ps_s.tile([64, QCH], FP32, tag="s")
                    nc.tensor.matmul(
                        psb[:],
                        lhsT=ones_l[:],
                        rhs=rec[:],
                        start=True,
                        stop=True,
                    )
                    bcs = bcp.tile([64, QCH], FP32, tag="bcs")
                    nc.vector.tensor_copy(bcs[:], psb[:])
                    kt2 = h // 2
                    lo = (h % 2) * 64
                    nc.vector.tensor_mul(
                        aT[lo:lo + 64, kt2 * N + c * QCH: kt2 * N + (c + 1) * QCH],
                        acc[0:64, :],
                        bcs[:],
                    )

        # ---- out projection: out[qi, :] = aT.T @ Wo_rows ----
        for it in range(N // P):
            for nh in range(2):
                pso = ps_main.tile([P, QCH], FP32, tag="proj")
                for kt in range(2):
                    nc.tensor.matmul(
                        pso[:],
                        lhsT=aT[:, kt * N + it * P: kt * N + (it + 1) * P],
                        rhs=wos[:, kt * D + nh * QCH: kt * D + (nh + 1) * QCH],
                        start=(kt == 0),
                        stop=(kt == 1),
                    )
                osb = pexp.tile([P, QCH], FP32, tag="osb", bufs=3)
                nc.vector.tensor_copy(osb[:], pso[:])
                nc.sync.dma_start(
                    out_d[it * P:(it + 1) * P, nh * QCH:(nh + 1) * QCH], osb[:]
                )

    nc.compile()
    return nc


_CACHE: dict = {}


def _module(biased: bool):
    if biased not in _CACHE:
        _CACHE[biased] = _build_module(biased)
    return _CACHE[biased]


def _mask_tiles():
    t = np.arange(4)[:, None, None]
    p = np.arange(P)[None, :, None]
    q = np.arange(QCH)[None, None, :]
    return (p + P * t <= q).astype(BF16NP).reshape(4 * P, QCH)


def kernel(x, context, Wq, bq, Wkv, bkv, Wo, bo, mask, context_mask):
    assert bool(np.all(mask)) and bool(np.all(context_mask)), (
        "only all-true padding masks are supported"
    )
    x = np.asarray(x, np.float32)
    context = np.asarray(context, np.float32)
    Wq, bq = np.asarray(Wq, np.float32), np.asarray(bq, np.float32)
    Wkv, bkv = np.asarray(Wkv, np.float32), np.asarray(bkv, np.float32)
    Wo, bo = np.asarray(Wo, np.float32), np.asarray(bo, np.float32)

    biased = bool(np.any(bq) or np.any(bkv))
    nc = _module(biased)

    msk = _mask_tiles()
    xkvT = [
        np.ascontiguousarray(
            np.concatenate([x[b], context[b]], axis=0).T.astype(BF16NP)
        )
        for b in range(B)
    ]
    in_maps = []
    for core in range(NCORES):
        b, g = divmod(core, GROUPS)
        cols = slice(g * GC, (g + 1) * GC)
        im = {
            "xkvT": xkvT[b],
            "wq": (Wq[:, cols] * SCALE).astype(BF16NP),
            "wk": Wkv[:, cols].astype(BF16NP),
            "wv": Wkv[:, D + g * GC: D + (g + 1) * GC].astype(BF16NP),
            "wo": np.ascontiguousarray(Wo[cols, :]).astype(BF16NP),
            "msk": msk,
        }
        if biased:
            im["bq"] = (bq[cols] * SCALE).astype(BF16NP).reshape(1, GC)
            im["bk"] = bkv[cols].astype(BF16NP).reshape(1, GC)
            im["bv"] = bkv[D + g * GC: D + (g + 1) * GC].astype(BF16NP).reshape(1, GC)
        in_maps.append(im)

    res = run_bass_kernel_spmd(nc, in_maps, core_ids=list(range(NCORES)))
    kernel.last_results = res
    out = np.zeros((B, N, D), np.float32)
    for core in range(NCORES):
        b = core // GROUPS
        out[b] += res.results[core]["out"]
    out += bo
    return out


# revision 10
# speedup vs baseline: 1.3711x; 1.3711x over previous
"""Trainium2 Bass kernel for DecoderAttention (b=2, n=2048, m=1024, d=1024, h=16).

Sharding: 8 cores = 2 (batch) x 4 (head groups of 4 heads).  Each core:
  - projects q/k/v for its 4 heads from x|context (pre-transposed on host),
  - runs causal flash attention in scores-transposed layout [kj, qi]
    (softmax without max subtraction -- scores are bounded; causally masked
    entries multiply to exactly 0 after exp, matching exp(-50000)),
  - computes its partial out-projection  attn_out_g @ Wo[rows_g]  [2048, 1024].
Host sums the 4 head-group partials per batch (the "all-reduce") and adds bo.

All matmuls run in bf16 with f32 PSUM accumulation (validated ~0.4% rel err).
"""

import os

# The neuron/axon jax backend must be discoverable for the PJRT execution
# path; a JAX_PLATFORMS=cpu pin (used when running the jax reference) would
# hide the trn2 devices from this process.
if os.environ.get("JAX_PLATFORMS", "").strip().lower() == "cpu":
    del os.environ["JAX_PLATFORMS"]

from contextlib import ExitStack

import ml_dtypes
import numpy as np

import concourse.bass as bass
import concourse.tile as tile
from concourse import bacc, mybir
from concourse.bass_utils import run_bass_kernel_spmd

B, N, M, D = 2, 2048, 1024, 1024
H, DH = 16, 64
NM = N + M          # 3072 keys (self + context)
GROUPS = 4          # head groups; 4 heads = 256 cols per group
GC = 256            # columns per head group
NCORES = 8
SCALE = DH ** -0.5
P = 128
KT = D // P         # 8 contraction tiles over d
QCH = 512           # query-chunk width
NQC = N // QCH      # 4 query chunks
NKJ = NM // P       # 24 key tiles
NSELF = N // P      # 16 self key tiles
FP32 = mybir.dt.float32
F32R = mybir.dt.float32r
BF16 = mybir.dt.bfloat16
BF16NP = ml_dtypes.bfloat16


def _active_kj(c):
    """Key tiles with any unmasked entry for query chunk c (512 queries)."""
    return list(range(0, 4 * c + 4)) + list(range(NSELF, NKJ))


def _build_module(biased: bool):
    nc = bacc.Bacc(
        "TRN2",
        target_bir_lowering=False,
        debug=False,
        enable_asserts=False,
        num_devices=NCORES,
    )
    xkvT_d = nc.dram_tensor("xkvT", [D, NM], BF16, kind="ExternalInput").ap()
    wq_d = nc.dram_tensor("wq", [D, GC], BF16, kind="ExternalInput").ap()
    wk_d = nc.dram_tensor("wk", [D, GC], BF16, kind="ExternalInput").ap()
    wv_d = nc.dram_tensor("wv", [D, GC], BF16, kind="ExternalInput").ap()
    wo_d = nc.dram_tensor("wo", [GC, D], BF16, kind="ExternalInput").ap()
    msk_d = nc.dram_tensor("msk", [4 * P, QCH], BF16, kind="ExternalInput").ap()
    if biased:
        bq_d = nc.dram_tensor("bq", [1, GC], BF16, kind="ExternalInput").ap()
        bk_d = nc.dram_tensor("bk", [1, GC], BF16, kind="ExternalInput").ap()
        bv_d = nc.dram_tensor("bv", [1, GC], BF16, kind="ExternalInput").ap()
    out_d = nc.dram_tensor("out", [N, D], FP32, kind="ExternalOutput").ap()

    with tile.TileContext(nc) as tc, ExitStack() as ctx:
        const = ctx.enter_context(tc.tile_pool(name="const", bufs=1))
        pexp = ctx.enter_context(tc.tile_pool(name="pexp", bufs=4))
        bcp = ctx.enter_context(tc.tile_pool(name="bcp", bufs=3))
        # PSUM budget: 8 banks = proj(1) + bc(1) + scores(2x2) + av(2)
        ps_main = ctx.enter_context(tc.tile_pool(name="ps_main", bufs=1, space="PSUM"))
        ps_s = ctx.enter_context(tc.tile_pool(name="ps_s", bufs=2, space="PSUM"))
        ps_av = ctx.enter_context(tc.tile_pool(name="ps_av", bufs=2, space="PSUM"))

        # ---- persistent SBUF tensors (column-concatenated k-tiles) ----
        xk = const.tile([P, KT * NM], BF16)          # xkvT: 8 tiles of [128, 3072]
        wqs = const.tile([P, KT * GC], BF16)
        wks = const.tile([P, KT * GC], BF16)
        wvs = const.tile([P, KT * GC], BF16)
        wos = const.tile([P, 2 * D], BF16)           # Wo rows: 2 tiles of [128, 1024]
        mks = const.tile([P, 4 * QCH], BF16)         # 4 diagonal mask tiles
        qT = const.tile([P, 2 * N], BF16)            # [head-pair cols, qi]
        kT = const.tile([P, 2 * NM], BF16)           # [head-pair cols, kj]
        vv = const.tile([P, NKJ * 4 * 65], BF16)     # per kj tile: 4x [v(64)|1]
        aT = const.tile([P, 2 * N], BF16)            # attn_out^T, 2 k-tiles
        ones_l = const.tile([1, 64], FP32)
        if biased:
            bq_s = const.tile([1, GC], BF16)
            bk_s = const.tile([1, GC], BF16)
            bv_s = const.tile([1, GC], BF16)
            ones_row = const.tile([1, QCH], BF16)
            ones_col = const.tile([1, P], BF16)

        # ---- input DMAs ----
        for kt in range(KT):
            nc.sync.dma_start(
                xk[:, kt * NM:(kt + 1) * NM], xkvT_d[kt * P:(kt + 1) * P, :]
            )
            nc.sync.dma_start(
                wqs[:, kt * GC:(kt + 1) * GC], wq_d[kt * P:(kt + 1) * P, :]
            )
            nc.sync.dma_start(
                wks[:, kt * GC:(kt + 1) * GC], wk_d[kt * P:(kt + 1) * P, :]
            )
            nc.sync.dma_start(
                wvs[:, kt * GC:(kt + 1) * GC], wv_d[kt * P:(kt + 1) * P, :]
            )
        for kt in range(2):
            nc.sync.dma_start(wos[:, kt * D:(kt + 1) * D], wo_d[kt * P:(kt + 1) * P, :])
        nc.sync.dma_start(
            mks.rearrange("p (t q) -> p t q", t=4),
            msk_d.rearrange("(t p) q -> p t q", p=P),
        )
        nc.vector.memset(ones_l[:], 1.0)
        if biased:
            nc.sync.dma_start(bq_s[:], bq_d[:])
            nc.sync.dma_start(bk_s[:], bk_d[:])
            nc.sync.dma_start(bv_s[:], bv_d[:])
            nc.vector.memset(ones_row[:], 1.0)
            nc.vector.memset(ones_col[:], 1.0)
        # ones columns interleaved into vv: col (t*260 + h*65 + 64)
        nc.gpsimd.memset(
            vv.rearrange("p (t h x) -> p t h x", t=NKJ, h=4)[:, :, :, 64:65], 1.0
        )

        # ---- emission helpers ----
        def emit_qT_group(mt, c):
            psq = ps_main.tile([P, QCH], FP32, tag="proj", name="psq")
            for kt in range(KT):
                nc.tensor.matmul(
                    psq[:],
                    lhsT=wqs[:, kt * GC + mt * P: kt * GC + (mt + 1) * P],
                    rhs=xk[:, kt * NM + c * QCH: kt * NM + (c + 1) * QCH],
                    start=(kt == 0),
                    stop=(kt == KT - 1) and not biased,
                )
            if biased:
                nc.tensor.matmul(
                    psq[:], lhsT=bq_s[:, mt * P:(mt + 1) * P], rhs=ones_row[:],
                    start=False, stop=True,
                )
            nc.vector.tensor_copy(
                qT[:, mt * N + c * QCH: mt * N + (c + 1) * QCH], psq[:]
            )

        def emit_kT_group(mt, c2):
            psk = ps_main.tile([P, QCH], FP32, tag="proj", name="psk")
            for kt in range(KT):
                nc.tensor.matmul(
                    psk[:],
                    lhsT=wks[:, kt * GC + mt * P: kt * GC + (mt + 1) * P],
                    rhs=xk[:, kt * NM + c2 * QCH: kt * NM + (c2 + 1) * QCH],
                    start=(kt == 0),
                    stop=(kt == KT - 1) and not biased,
                )
            if biased:
                nc.tensor.matmul(
                    psk[:], lhsT=bk_s[:, mt * P:(mt + 1) * P], rhs=ones_row[:],
                    start=False, stop=True,
                )
            nc.vector.tensor_copy(
                kT[:, mt * NM + c2 * QCH: mt * NM + (c2 + 1) * QCH], psk[:]
            )

        def emit_v_group(t):
            psv = ps_main.tile([P, GC], FP32, tag="proj", name="psv")
            for kt in range(KT):
                nc.tensor.matmul(
                    psv[:],
                    lhsT=xk[:, kt * NM + t * P: kt * NM + (t + 1) * P],
                    rhs=wvs[:, kt * GC:(kt + 1) * GC],
                    start=(kt == 0),
                    stop=(kt == KT - 1) and not biased,
                )
            if biased:
                nc.tensor.matmul(
                    psv[:], lhsT=ones_col[:], rhs=bv_s[:], start=False, stop=True,
                )
            nc.vector.tensor_copy(
                vv[:, t * 260:(t + 1) * 260].rearrange("p (h x) -> p h x", h=4)[
                    :, :, 0:64
                ],
                psv.rearrange("p (h x) -> p h x", h=4),
            )

        def emit_attention_chunk(c):
            kjs = _active_kj(c)
            last = len(kjs) - 1
            for pair in range(2):
                ps_acc = [None, None]
                pending = None  # (p_tile, i) exp'd tiles not yet fed to AV

                def do_av(pt, i):
                    t = kjs[i]
                    for hh in range(2):
                        h = pair * 2 + hh
                        nc.tensor.matmul(
                            ps_acc[hh][:],
                            lhsT=vv[:, t * 260 + h * 65: t * 260 + (h + 1) * 65],
                            rhs=pt[:, hh * QCH:(hh + 1) * QCH],
                            start=(i == 0),
                            stop=(i == last),
                        )

                for i, t in enumerate(kjs):
                    # both heads' scores into one 2-bank psum tile
                    pss = ps_s.tile([P, 2 * QCH], FP32, tag="s", name="pss")
                    for hh in range(2):
                        lo, hi = hh * 64, hh * 64 + 64
                        nc.tensor.matmul(
                            pss[:, hh * QCH:(hh + 1) * QCH],
                            lhsT=kT[lo:hi, pair * NM + t * P: pair * NM + (t + 1) * P],
                            rhs=qT[lo:hi, pair * N + c * QCH: pair * N + (c + 1) * QCH],
                            start=True,
                            stop=True,
                        )
                    pt = pexp.tile([P, 2 * QCH], BF16, tag="p", name="pt")
                    nc.scalar.activation(
                        pt[:], pss[:], mybir.ActivationFunctionType.Exp
                    )
                    if 4 * c <= t < 4 * c + 4:  # diagonal tile: causal mask
                        dt = t - 4 * c
                        for hh in range(2):
                            nc.vector.tensor_mul(
                                pt[:, hh * QCH:(hh + 1) * QCH],
                                pt[:, hh * QCH:(hh + 1) * QCH],
                                mks[:, dt * QCH:(dt + 1) * QCH],
                            )
                    if i == 0:
                        ps_acc[0] = ps_av.tile([65, QCH], FP32, tag="av", name="av0")
                        ps_acc[1] = ps_av.tile([65, QCH], FP32, tag="av", name="av1")
                    if pending is not None:
                        do_av(*pending)
                    pending = (pt, i)
                do_av(*pending)

                # normalize: rows 0..63 / row 64, write into aT
                for hh in range(2):
                    h = pair * 2 + hh
                    acc = ps_acc[hh]
                    rec = bcp.tile([1, QCH], FP32, tag="rec", name="rec")
                    nc.vector.reciprocal(rec[:], acc[64:65, :])
                    psb = ps_main.tile([64, QCH], FP32, tag="bc", name="psb")
                    nc.tensor.matmul(
                        psb[:], lhsT=ones_l[:], rhs=rec[:], start=True, stop=True,
                    )
                    bcs = bcp.tile([64, QCH], FP32, tag="bcs", name="bcs")
                    nc.vector.tensor_copy(bcs[:], psb[:])
                    kt2 = h // 2
                    lo = (h % 2) * 64
                    nc.vector.tensor_mul(
                        aT[lo:lo + 64, kt2 * N + c * QCH: kt2 * N + (c + 1) * QCH],
                        acc[0:64, :],
                        bcs[:],
                    )

        def emit_outproj_chunk(c):
            for it in range(4 * c, 4 * c + 4):
                for nh in range(2):
                    pso = ps_main.tile([P, QCH], FP32, tag="proj", name="pso")
                    for kt in range(2):
                        nc.tensor.matmul(
                            pso[:],
                            lhsT=aT[:, kt * N + it * P: kt * N + (it + 1) * P],
                            rhs=wos[:, kt * D + nh * QCH: kt * D + (nh + 1) * QCH],
                            start=(kt == 0),
                            stop=(kt == 1),
                        )
                    osb = pexp.tile([P, QCH], FP32, tag="osb", bufs=3, name="osb")
                    nc.vector.tensor_copy(osb[:], pso[:])
                    nc.sync.dma_start(
                        out_d[it * P:(it + 1) * P, nh * QCH:(nh + 1) * QCH], osb[:]
                    )

        # ---- interleaved emission: projections feed the ACT-bound attention
        # phase as TensorE filler; out-projection trails each chunk ----
        for mt in range(2):
            for c in range(NQC):
                emit_qT_group(mt, c)
        for c2 in (0, 4, 5):
            for mt in range(2):
                emit_kT_group(mt, c2)
        for t in list(range(0, 4)) + list(range(NSELF, NKJ)):
            emit_v_group(t)
        for c in range(NQC):
            emit_attention_chunk(c)
            if c < NQC - 1:
                for mt in range(2):
                    emit_kT_group(mt, c + 1)
                for t in range(4 * (c + 1), 4 * (c + 1) + 4):
                    emit_v_group(t)
            emit_outproj_chunk(c)

    nc.compile()
    return nc


_CACHE: dict = {}


def _module(biased: bool):
    if biased not in _CACHE:
        _CACHE[biased] = _build_module(biased)
    return _CACHE[biased]


def _mask_tiles():
    t = np.arange(4)[:, None, None]
    p = np.arange(P)[None, :, None]
    q = np.arange(QCH)[None, None, :]
    return (p + P * t <= q).astype(BF16NP).reshape(4 * P, QCH)


def kernel(x, context, Wq, bq, Wkv, bkv, Wo, bo, mask, context_mask):
    assert bool(np.all(mask)) and bool(np.all(context_mask)), (
        "only all-true padding masks are supported"
    )
    x = np.asarray(x, np.float32)
    context = np.asarray(context, np.float32)
    Wq, bq = np.asarray(Wq, np.float32), np.asarray(bq, np.float32)
    Wkv, bkv = np.asarray(Wkv, np.float32), np.asarray(bkv, np.float32)
    Wo, bo = np.asarray(Wo, np.float32), np.asarray(bo, np.float32)

    biased = bool(np.any(bq) or np.any(bkv))
    nc = _module(biased)

    msk = _mask_tiles()
    xkvT = [
        np.ascontiguousarray(
            np.concatenate([x[b], context[b]], axis=0).T.astype(BF16NP)
        )
        for b in range(B)
    ]
    in_maps = []
    for core in range(NCORES):
        b, g = divmod(core, GROUPS)
        cols = slice(g * GC, (g + 1) * GC)
        im = {
            "xkvT": xkvT[b],
            "wq": (Wq[:, cols] * SCALE).astype(BF16NP),
            "wk": Wkv[:, cols].astype(BF16NP),
            "wv": Wkv[:, D + g * GC: D + (g + 1) * GC].astype(BF16NP),
            "wo": np.ascontiguousarray(Wo[cols, :]).astype(BF16NP),
            "msk": msk,
        }
        if biased:
            im["bq"] = (bq[cols] * SCALE).astype(BF16NP).reshape(1, GC)
            im["bk"] = bkv[cols].astype(BF16NP).reshape(1, GC)
            im["bv"] = bkv[D + g * GC: D + (g + 1) * GC].astype(BF16NP).reshape(1, GC)
        in_maps.append(im)

    res = run_bass_kernel_spmd(nc, in_maps, core_ids=list(range(NCORES)))
    kernel.last_results = res
    out = np.zeros((B, N, D), np.float32)
    for core in range(NCORES):
        b = core // GROUPS
        out[b] += res.results[core]["out"]
    out += bo
    return out
